# revision 25
# baseline (speedup 1.0000x reference)
"""Trainium2 Bass kernel: parity-polynomial segment_reduce.

Reference math:
    spins = 1 - 2*bits                                   # {-1,+1}
    parities[b,t] = prod_o spins_pad[b, idx_pad[t,o]]    # [B, T]
    out[b] = parities[b] @ theta

Every parity factor is (-1)^{bit}, so
    out[b] = sum_t theta[t] * (-1)^{popcount(key[b] & mask[t])}
with key[b] = sum_i bits[b,i]<<i and mask[t] = XOR-fold of (1<<idx_pad[t,o])
(the pad index NUM_BITS maps to a constant-one column; a repeated index
squares to +1, which XOR-folding reproduces).

idx_pad only references bits 0..11, so every mask < 4096 and
out[b] = f(key12[b]) where f = WHT_4096(theta_spread), a 4096-point
Walsh-Hadamard transform of theta scattered by mask.  On device (per core,
batch-sharded 512 rows), with the 64x64 Kronecker split
H_4096 = H_64 (x) H_64  (p = key>>6, c = key&63):

  1. G[q,c]   = (Theta @ H64)[q,c],  Theta[q,d] = ts[64q+d]      (matmul)
  2. F_cm[c,p]= (G^T-contracted) sum_q G[q,c] H64[q,p]           (matmul)
  3. One key matmul produces BOTH selector residues stacked in 128
     partitions: rows j<64 get p_key(b)-j, rows j>=64 get c_key(b)-(j-64)
     (bitsT carries a constant-ones row; all operands bf16-exact ints).
  4. One is_equal(.,0) gives both one-hots [128,512]: ohp=rows 0:64,
     ohc=rows 64:128.
  5. Gather + mask + reduce, split into two 256-column batch halves so
     PE and DVE pipeline (PE: g1,g2,s1,s2; DVE: oh1,oh2,m1,m2,c1,c2 --
     the DVE runs saturated, which sets the tail length):
        o1[p,b]  = sum_c F_cm[c,p] ohc[c,b]      (matmul)   = F[p, c_b]
        pr[p,b]  = o1[p,b] * ohp[p,b]            (DVE)
        out[b]   = sum_p pr[p,b]                 (ones matmul)

Everything is bf16 on the PE (single-pass matmuls; fp32 runs 2-pass),
fp32 PSUM accumulate.  Verified rel err ~3e-3 vs fp64 (gate 2e-2).

Raw bacc (no TileContext): the kernel is ~15 instructions, so manual
semaphores cost nothing and skip the Tile scheduler's extra
barrier/ordering preamble and drain/release epilogue.  The two input
DMAs are dispatched on the two independent HWDGE rings (SP via
nc.sync, ACT via nc.scalar) so their dispatch costs overlap; each PSUM
tensor/half gets its own 2KB bank (8 tensors = all 8 banks) so PE
writes and DVE reads never collide.
The ones-column for the final reduce is memset by the otherwise-idle
Pool engine at base partition 64, which keeps every operand of the
tail ops at a uniform base partition (matmul requires equal input
bases; the gather therefore lands its output at PSUM partitions
64:128).  The F cast runs on the ACT engine so the DVE's is_equal
never blocks it; a dummy ACT copy right after the wt dispatch pulls
the one-time activation-table load into the dead DMA-wait window.
Semaphores are single-producer (the race checker forbids two engines
incrementing one semaphore), so the gather joins DVE work (vsem) and
the ACT cast (asem) with a two-wait event-semaphore.

Host does only sharding, dtype/layout staging, and index bookkeeping
(mask XOR-fold + theta scatter).  All theta- and bit-dependent
arithmetic runs on device.
"""

import numpy as np

B, NUM_BITS, ORDER = 4096, 32, 12
N_CORES = 8
B_LOCAL = B // N_CORES          # 512
KEYS = 1 << ORDER               # 4096
P_DIM, C_DIM = 64, 64           # KEYS = P_DIM * C_DIM ; p = key>>6, c = key&63
ROWS = ORDER + 1                # 12 bit rows + constant-ones row
BB_COLS = B_LOCAL + 128         # bitsT | wpc
WT_COLS = 64 + 64               # thetaT | h64

_STATE = {}


def _sylvester(n):
    """H[i,j] = (-1)^popcount(i&j), Sylvester ordering."""
    h = np.array([[1.0]], dtype=np.float32)
    while h.shape[0] < n:
        h = np.block([[h, h], [h, -h]])
    return np.ascontiguousarray(h, dtype=np.float32)


def _build_module():
    import concourse.mybir as mybir
    from concourse import bacc

    f32 = mybir.dt.float32
    bf16 = mybir.dt.bfloat16
    nc = bacc.Bacc(
        "TRN2",
        target_bir_lowering=False,
        debug=False,
        enable_asserts=False,
        num_devices=N_CORES,
    )

    bb = nc.dram_tensor("bb", [ROWS, BB_COLS], bf16, kind="ExternalInput").ap()
    wt = nc.dram_tensor("wt", [64, WT_COLS], bf16, kind="ExternalInput").ap()
    out = nc.dram_tensor("out", [1, B_LOCAL], f32, kind="ExternalOutput").ap()

    t_wt = nc.alloc_sbuf_tensor("t_wt", [64, WT_COLS], bf16).ap()
    t_on = nc.alloc_sbuf_tensor("t_on", [128, 1], bf16).ap()
    t_scr = nc.alloc_sbuf_tensor("t_scr", [128, 1], f32).ap()
    t_bb = nc.alloc_sbuf_tensor("t_bb", [ROWS, BB_COLS], bf16).ap()
    t_G = nc.alloc_sbuf_tensor("t_G", [64, 64], bf16).ap()
    t_oh = nc.alloc_sbuf_tensor("t_oh", [128, B_LOCAL], bf16).ap()
    t_F = nc.alloc_sbuf_tensor("t_F", [64, 64], bf16).ap()
    t_pr = nc.alloc_sbuf_tensor("t_pr", [128, B_LOCAL], bf16).ap()
    t_out = nc.alloc_sbuf_tensor("t_out", [1, B_LOCAL], f32).ap()

    # One PSUM bank (2KB/partition) per tensor/half so PE-writes and
    # DVE-reads never share a bank: 8 tensors = all 8 banks.
    p_G = nc.alloc_psum_tensor("p_G", [128, 512], f32).ap()
    p_K1 = nc.alloc_psum_tensor("p_K1", [128, 512], f32).ap()
    p_K2 = nc.alloc_psum_tensor("p_K2", [128, 512], f32).ap()
    p_F = nc.alloc_psum_tensor("p_F", [128, 512], f32).ap()
    p_o1 = nc.alloc_psum_tensor("p_o1", [128, 512], f32).ap()
    p_o2 = nc.alloc_psum_tensor("p_o2", [128, 512], f32).ap()
    p_u1 = nc.alloc_psum_tensor("p_u1", [128, 512], f32).ap()
    p_u2 = nc.alloc_psum_tensor("p_u2", [128, 512], f32).ap()

    gsem = nc.alloc_semaphore("gsem")
    asem = nc.alloc_semaphore("asem")
    wsem = nc.alloc_semaphore("wsem")
    bsem = nc.alloc_semaphore("bsem")
    psem = nc.alloc_semaphore("psem")
    vsem = nc.alloc_semaphore("vsem")
    osem = nc.alloc_semaphore("osem")

    t_thetaT = t_wt[:, 0:64]
    t_h64 = t_wt[:, 64:128]
    t_ones = t_on[64:128, :]                # base partition 64
    t_bitsT = t_bb[:, 0:B_LOCAL]
    t_wpc = t_bb[:, B_LOCAL : B_LOCAL + 128]

    # input DMAs on the two independent HWDGE rings, dispatched as soon as
    # each engine clears the bass init barrier
    nc.scalar.dma_start(out=t_wt, in_=wt).then_inc(wsem, 16)
    nc.sync.dma_start(out=t_bb, in_=bb).then_inc(bsem, 16)
    # ones column for the final reduce, at base partition 64 (Pool is idle)
    nc.gpsimd.memset(t_ones, 1.0).then_inc(gsem, 1)

    # PE stream: G, K1, F, K2, g1, g2, s1, s2
    nc.tensor.wait_ge(gsem, 1)
    nc.tensor.wait_ge(wsem, 16)
    nc.tensor.matmul(p_G[0:64, 0:64], t_thetaT, t_h64).then_inc(psem, 1)
    nc.tensor.wait_ge(bsem, 16)
    nc.tensor.matmul(p_K1[:, 0:256], t_wpc, t_bitsT[:, 0:256]).then_inc(psem, 1)
    nc.tensor.wait_ge(vsem, 1)
    nc.tensor.matmul(p_F[0:64, 0:64], t_G, t_h64).then_inc(psem, 1)
    nc.tensor.matmul(p_K2[:, 0:256], t_wpc, t_bitsT[:, 256:512]).then_inc(psem, 1)
    nc.tensor.wait_ge(asem, 1)
    nc.tensor.wait_ge(vsem, 2)
    nc.tensor.matmul(p_o1[64:128, 0:256], t_F, t_oh[0:64, 0:256]).then_inc(psem, 1)
    nc.tensor.wait_ge(vsem, 3)
    nc.tensor.matmul(p_o2[64:128, 0:256], t_F, t_oh[0:64, 256:512]).then_inc(psem, 1)
    nc.tensor.wait_ge(vsem, 4)
    nc.tensor.matmul(p_u1[0:1, 0:256], t_ones, t_pr[64:128, 0:256]).then_inc(psem, 1)
    nc.tensor.wait_ge(vsem, 5)
    nc.tensor.matmul(p_u2[0:1, 0:256], t_ones, t_pr[64:128, 256:512]).then_inc(psem, 1)

    # DVE stream: Gcast, oh1, oh2, mul1, mul2, copy1, copy2
    nc.vector.wait_ge(psem, 1)
    nc.vector.tensor_copy(t_G, p_G[0:64, 0:64]).then_inc(vsem, 1)
    nc.vector.wait_ge(psem, 2)
    nc.vector.tensor_scalar(
        out=t_oh[:, 0:256],
        in0=p_K1[:, 0:256],
        scalar1=0.0,
        scalar2=None,
        op0=mybir.AluOpType.is_equal,
    ).then_inc(vsem, 1)
    nc.vector.wait_ge(psem, 4)
    nc.vector.tensor_scalar(
        out=t_oh[:, 256:512],
        in0=p_K2[:, 0:256],
        scalar1=0.0,
        scalar2=None,
        op0=mybir.AluOpType.is_equal,
    ).then_inc(vsem, 1)
    nc.vector.wait_ge(psem, 5)
    nc.vector.tensor_mul(
        t_pr[64:128, 0:256], p_o1[64:128, 0:256], t_oh[64:128, 0:256]
    ).then_inc(vsem, 1)
    nc.vector.wait_ge(psem, 6)
    nc.vector.tensor_mul(
        t_pr[64:128, 256:512], p_o2[64:128, 0:256], t_oh[64:128, 256:512]
    ).then_inc(vsem, 1)
    nc.vector.wait_ge(psem, 7)
    nc.vector.tensor_copy(t_out[:, 0:256], p_u1[0:1, 0:256]).then_inc(vsem, 1)
    nc.vector.wait_ge(psem, 8)
    nc.vector.tensor_copy(t_out[:, 256:512], p_u2[0:1, 0:256]).then_inc(vsem, 1)

    # ACT: dummy activation right after the wt dispatch so the Copy-func
    # table load lands in the dead DMA-wait window, not before the F cast
    nc.scalar.copy(t_scr, nc.const_aps.aps[(f32, 0.0)])
    # ACT: F cast off the DVE critical path
    nc.scalar.wait_ge(psem, 3)
    nc.scalar.copy(t_F, p_F[0:64, 0:64]).then_inc(asem, 1)

    # output DMA (ACT ring: its HWDGE FIFO is idle after wt).  No kernel-side
    # receipt wait: NRT quiesces pending DMA rings at inference end, and the
    # write-receipt sits ~2us after the last engine instruction otherwise.
    nc.scalar.wait_ge(vsem, 7)
    nc.scalar.dma_start(out=out, in_=t_out).then_inc(osem, 16)

    nc.compile()
    return nc


def _get_module():
    nc = _STATE.get("nc")
    if nc is None:
        nc = _build_module()
        _STATE["nc"] = nc
    return nc


def _host_prep(bitstrings, theta, idx_pad):
    """Index bookkeeping + input staging. Returns per-core input maps."""
    import ml_dtypes

    bf16 = ml_dtypes.bfloat16
    bitstrings = np.asarray(bitstrings)
    theta = np.asarray(theta, dtype=np.float32)
    idx_pad = np.asarray(idx_pad).astype(np.int64)

    # mask[t] = XOR-fold of one-hot bit positions (pad index >= NUM_BITS -> no bit)
    onehots = np.where(idx_pad >= NUM_BITS, 0, np.int64(1) << np.clip(idx_pad, 0, 62))
    masks = np.bitwise_xor.reduce(onehots, axis=1)
    if masks.size and int(masks.max()) >= KEYS:
        raise NotImplementedError(
            "kernel specialized for masks spanning bits 0..11 "
            f"(max mask {int(masks.max())})"
        )
    theta_spread = np.zeros(KEYS, np.float32)
    np.add.at(theta_spread, masks, theta)

    # Stationary selector weights: col j<64 -> c_key - j, col j>=64 -> p_key - (j-64)
    wpc = np.zeros((ROWS, 128), np.float32)
    for k in range(6):
        wpc[k, 0:64] = float(1 << k)
    for k in range(6, ORDER):
        wpc[k, 64:128] = float(1 << (k - 6))
    wpc[ORDER, 0:64] = -np.arange(64, dtype=np.float32)
    wpc[ORDER, 64:128] = -np.arange(64, dtype=np.float32)

    wt = np.zeros((64, WT_COLS), np.float32)
    wt[:, 0:64] = theta_spread.reshape(64, 64).T      # thetaT[d,q] = ts[64q+d]
    wt[:, 64:128] = _sylvester(64)

    base = {"wt": wt.astype(bf16)}

    bits_f = bitstrings[:, :ORDER].astype(np.float32)
    in_maps = []
    for c in range(N_CORES):
        m = dict(base)
        bbuf = np.ones((ROWS, BB_COLS), np.float32)
        bbuf[:ORDER, 0:B_LOCAL] = bits_f[c * B_LOCAL : (c + 1) * B_LOCAL, :].T
        bbuf[:, B_LOCAL:] = wpc
        m["bb"] = bbuf.astype(bf16)
        in_maps.append(m)
    return in_maps


def kernel(bitstrings, theta, idx_pad):
    from concourse.bass_utils import run_bass_kernel_spmd

    in_maps = _host_prep(bitstrings, theta, idx_pad)
    nc = _get_module()
    res = run_bass_kernel_spmd(nc, in_maps, core_ids=list(range(N_CORES)))
    out = np.concatenate([np.asarray(r["out"][0]) for r in res.results])
    return out.astype(np.float32)
